# revision 1
# baseline (speedup 1.0000x reference)
"""Trainium2 Bass kernel for BinarizedLinear + BatchNorm (training-mode, affine=False).

Computes: y = BN(sign(x) @ sign(W).T + bias), BN over the batch axis with
biased variance. bias is ignored: BN subtracts the batch mean, which absorbs
any per-feature constant exactly.

Sharding: data-parallel over the batch (B/8 rows per core). Each core also
owns 1/8 of W's rows (one 512-feature chunk): it binarizes them to fp8,
transposes to K-major on-chip, and an AllGather distributes the packed
transposed chunks to every core. Matmuls run in fp8 with DoubleRow perf mode
(two K-tiles per instruction). BN batch stats (per-feature sum / sum of
squares) use one small AllReduce per 512-feature chunk, pipelined one chunk
behind the matmuls so the PE never waits on a collective.

Numerics: sign values (+-1) are exact in fp8e4/bf16, products are +-1 and
accumulation happens in fp32 PSUM, so the matmul is exact. Raw outputs are
sums of IN odd terms -> even integers with |out| <= IN, exactly representable
in fp16 (integers to 2048, even integers to 4096).
"""

import numpy as np

import concourse.bass as bass
import concourse.mybir as mybir
import concourse.tile as tile
from concourse import bacc
from concourse.bass_utils import run_bass_kernel_spmd
from concourse.masks import make_identity

P = 128
BN_EPS = 1e-5

F32 = mybir.dt.float32
BF16 = mybir.dt.bfloat16
F16 = mybir.dt.float16
F8 = mybir.dt.float8e4


class Cfg:
    def __init__(self, B=8192, IN=4096, OUT=4096, n_cores=8, oc=512):
        assert OUT // oc == n_cores, "one output chunk per core"
        self.B, self.IN, self.OUT, self.n_cores = B, IN, OUT, n_cores
        self.B_SH = B // n_cores          # batch rows per core
        self.BT = self.B_SH // P          # batch tiles per core
        self.KT = IN // P                 # contraction (K) tiles
        self.OC = oc                      # output-feature chunk width (matmul N)
        self.NOC = OUT // oc              # number of output chunks (== n_cores)
        self.S = oc // P                  # W row-tiles per chunk
        self.WH = min(2048, IN)           # load half-width (free elems)
        self.NH = IN // self.WH           # loads per row-tile
        self.KH = self.WH // P            # K tiles per load


def build_program(cfg: Cfg, reps: int = 1):
    """Build the SPMD Bass program (same NEFF on every core)."""
    nc = bacc.Bacc(
        "TRN2",
        target_bir_lowering=False,
        debug=False,
        enable_asserts=False,
        num_devices=cfg.n_cores,
    )

    x_in = nc.dram_tensor("x_shard", [cfg.B_SH, cfg.IN], F32, kind="ExternalInput")
    w_in = nc.dram_tensor("w_slice", [cfg.OC, cfg.IN], F32, kind="ExternalInput")
    y_out = nc.dram_tensor("y", [cfg.B_SH, cfg.OUT], F32, kind="ExternalOutput")

    coll_space = "Shared" if cfg.n_cores > 4 else "Local"
    # packed K-major fp8 transposed W chunk: [p, (k, s, o')] dump layout
    CHW = cfg.KT * cfg.S * P  # packed columns per chunk
    ag_in = nc.dram_tensor("ag_in", [P, CHW], F8, kind="Internal")
    ag_out = nc.dram_tensor(
        "ag_out", [cfg.n_cores * P, CHW], F8, kind="Internal",
        addr_space=coll_space,
    )
    raw_d = nc.dram_tensor("raw", [cfg.B_SH, cfg.OUT], F16, kind="Internal")
    st_in = nc.dram_tensor("stats_in", [cfg.NOC, 2, cfg.OC], F32, kind="Internal")
    st_out = nc.dram_tensor(
        "stats_out", [cfg.NOC, 2, cfg.OC], F32, kind="Internal",
        addr_space=coll_space,
    )
    rows_d = nc.dram_tensor("rows", [cfg.NOC, 2, cfg.OC], F32, kind="Internal")

    groups = [list(range(cfg.n_cores))]

    with tile.TileContext(nc) as tc:
        with (
            tc.tile_pool(name="const", bufs=1) as const,
            tc.tile_pool(name="xt", bufs=1) as xtp,
            tc.tile_pool(name="osb", bufs=6) as osbp,
            tc.tile_pool(name="sq", bufs=3) as sqp,
            tc.tile_pool(name="stt", bufs=2) as sttp,
            tc.tile_pool(name="bc", bufs=2) as bcp,
            tc.tile_pool(name="norm", bufs=4) as normp,
            tc.tile_pool(name="psm", bufs=4, space="PSUM") as psm,
            tc.tile_pool(name="psst", bufs=1, space="PSUM") as psst,
            tc.tile_pool(name="pstp", bufs=2, space="PSUM") as pstp,
        ):
            ones_h = const.tile([P, 1], F16, tag="ones_h")
            nc.vector.memset(ones_h[:], 1.0)
            ones_f = const.tile([P, 1], F32, tag="ones_f")
            nc.vector.memset(ones_f[:], 1.0)
            eps_t = const.tile([P, 1], F32, tag="eps")
            nc.vector.memset(eps_t[:], float(BN_EPS))
            ident = const.tile([P, P], BF16, tag="ident")
            make_identity(nc, ident[:])

            pools = dict(
                xtp=xtp, osbp=osbp, sqp=sqp, sttp=sttp, bcp=bcp, normp=normp,
                psm=psm, psst=psst, pstp=pstp,
            )
            consts = dict(ones_h=ones_h, ones_f=ones_f, eps_t=eps_t, ident=ident)
            tensors = dict(
                x_in=x_in, w_in=w_in, y_out=y_out, raw_d=raw_d,
                ag_in=ag_in, ag_out=ag_out,
                st_in=st_in, st_out=st_out, rows_d=rows_d,
            )
            for _rep in range(reps):
                _emit_once(nc, tc, cfg, groups, tensors, pools, consts)

    nc.compile()
    return nc


def _emit_once(nc, tc, cfg, groups, T, pools, C):
    xtp, osbp, sqp, sttp, bcp, normp = (
        pools["xtp"], pools["osbp"], pools["sqp"], pools["sttp"],
        pools["bcp"], pools["normp"],
    )
    psm, psst, pstp = pools["psm"], pools["psst"], pools["pstp"]
    ones_h, ones_f, eps_t, ident = C["ones_h"], C["ones_f"], C["eps_t"], C["ident"]
    x_in, w_in, y_out = T["x_in"], T["w_in"], T["y_out"]
    raw_d, ag_in, ag_out = T["raw_d"], T["ag_in"], T["ag_out"]
    st_in, st_out, rows_d = T["st_in"], T["st_out"], T["rows_d"]
    inv_b = 1.0 / float(cfg.B)
    npair = cfg.KT // 2

    # xT[p, bt, k, b'] = sign(x)[bt*128+b', k*128+p]  (fp8, K-major)
    xt_t = xtp.tile([P, cfg.BT, cfg.KT, P], F8, tag="xt")

    with (
        tc.tile_pool(name="wstage", bufs=2) as wstage,
        tc.tile_pool(name="wbin", bufs=2) as wbin,
        tc.tile_pool(name="tmp8", bufs=2) as tmp8p,
        tc.tile_pool(name="slice8", bufs=1) as slice8p,
        tc.tile_pool(name="bwt", bufs=3) as bwtp,
    ):
        # ---- W slice prep: sign -> transpose -> fp8 K-major -> DRAM ----
        # slice8[p, k, s, o'] = sign(W_slice)[s*128 + o', k*128 + p]
        slice8 = slice8p.tile([P, cfg.KT, cfg.S, P], F8, tag="slice8")
        for s in range(cfg.S):
            wfs = []
            for h in range(cfg.NH):
                wf = wstage.tile([P, cfg.WH], F32, tag="wstage")
                eng = nc.sync if (s + h) % 2 == 0 else nc.scalar
                eng.dma_start(
                    wf[:],
                    w_in.ap()[s * P:(s + 1) * P, h * cfg.WH:(h + 1) * cfg.WH],
                )
                wfs.append(wf)
            for h in range(cfg.NH):
                wb = wbin.tile([P, cfg.WH], BF16, tag="wbin")
                nc.scalar.sign(wb[:], wfs[h][:])
                tmp = tmp8p.tile([P, cfg.KH, P], BF16, tag="tmp")
                nc.sync.dma_start(tmp[:], wb[:], transpose=True)
                nc.vector.tensor_copy(
                    slice8[:, h * cfg.KH:(h + 1) * cfg.KH, s, :], tmp[:]
                )
        nc.sync.dma_start(
            ag_in.ap()[:, :], slice8[:].rearrange("p a b c -> p (a b c)")
        )

        # ---- x prep: sign -> transpose -> fp8 ----
        for bt in range(cfg.BT):
            wfs = []
            for h in range(cfg.NH):
                wf = wstage.tile([P, cfg.WH], F32, tag="wstage")
                eng = nc.sync if (bt + h) % 2 == 0 else nc.scalar
                eng.dma_start(
                    wf[:],
                    x_in.ap()[bt * P:(bt + 1) * P,
                              h * cfg.WH:(h + 1) * cfg.WH],
                )
                wfs.append(wf)
            for h in range(cfg.NH):
                xb = wbin.tile([P, cfg.WH], BF16, tag="wbin")
                nc.scalar.sign(xb[:], wfs[h][:])
                for kl in range(cfg.KH):
                    tp = pstp.tile([P, P], BF16, tag="tp")
                    nc.tensor.transpose(
                        tp[:], xb[:, kl * P:(kl + 1) * P], ident[:]
                    )
                    nc.vector.tensor_copy(
                        xt_t[:, bt, h * cfg.KH + kl, :], tp[:]
                    )

        # ---- distribute packed transposed W chunks ----
        nc.gpsimd.collective_compute(
            "AllGather",
            mybir.AluOpType.bypass,
            replica_groups=groups,
            ins=[ag_in.ap().opt()],
            outs=[ag_out.ap().opt()],
        )

        def w_fetch(oc):
            bwt_tile = bwtp.tile([P, cfg.KT, cfg.S, P], F8, tag="bwt")
            nc.sync.dma_start(
                bwt_tile[:].rearrange("p a b c -> p (a b c)"),
                ag_out.ap()[oc * P:(oc + 1) * P, :],
            )
            return bwt_tile

        def matmuls(oc, bwt_tile):
            psum_sum = psst.tile([1, cfg.OC], F32, tag="pssum")
            psum_sq = psst.tile([1, cfg.OC], F32, tag="pssq")
            prev = None
            for bt in range(cfg.BT + 1):
                cur = None
                if bt < cfg.BT:
                    ps = psm.tile([P, cfg.OC], F32, tag="mm")
                    for i in range(npair):
                        nc.tensor.matmul(
                            ps[:],
                            xt_t[:, bt, 2 * i:2 * i + 2, :],
                            bwt_tile[:, 2 * i:2 * i + 2, :, :],
                            start=(i == 0),
                            stop=(i == npair - 1),
                            perf_mode=mybir.MatmulPerfMode.DoubleRow,
                        )
                    cur = (bt, ps)
                if prev is not None:
                    pbt, pps = prev
                    sq = sqp.tile([P, cfg.OC], F32, tag="sq")
                    nc.scalar.square(sq[:], pps[:])
                    ob = osbp.tile([P, cfg.OC], F16, tag="ob")
                    nc.vector.tensor_copy(ob[:], pps[:])
                    nc.tensor.matmul(
                        psum_sum[:], ones_h[:], ob[:],
                        start=(pbt == 0), stop=(pbt == cfg.BT - 1),
                    )
                    nc.tensor.matmul(
                        psum_sq[:], ones_f[:], sq[:],
                        start=(pbt == 0), stop=(pbt == cfg.BT - 1),
                    )
                    nc.sync.dma_start(
                        raw_d.ap()[
                            pbt * P:(pbt + 1) * P,
                            oc * cfg.OC:(oc + 1) * cfg.OC,
                        ],
                        ob[:],
                    )
                prev = cur
            return psum_sum, psum_sq

        def stats_copy(oc, psum_sum, psum_sq):
            srow = sttp.tile([1, cfg.OC], F32, tag="srow")
            nc.vector.tensor_copy(srow[:], psum_sum[:])
            qrow = sttp.tile([1, cfg.OC], F32, tag="qrow")
            nc.vector.tensor_copy(qrow[:], psum_sq[:])
            nc.sync.dma_start(st_in.ap()[oc, 0:1, :], srow[:])
            nc.sync.dma_start(st_in.ap()[oc, 1:2, :], qrow[:])

        def ar_pair(oc0, n):
            nc.gpsimd.collective_compute(
                "AllReduce",
                mybir.AluOpType.add,
                replica_groups=groups,
                ins=[st_in.ap()[oc0:oc0 + n].opt()],
                outs=[st_out.ap()[oc0:oc0 + n].opt()],
            )

        def stats_math(oc):
            gsum = sttp.tile([1, cfg.OC], F32, tag="gsum")
            nc.gpsimd.dma_start(gsum[:], st_out.ap()[oc, 0:1, :])
            gsq = sttp.tile([1, cfg.OC], F32, tag="gsq")
            nc.gpsimd.dma_start(gsq[:], st_out.ap()[oc, 1:2, :])
            # mean, E[x^2]
            nc.vector.tensor_scalar_mul(gsum[:], gsum[:], inv_b)
            nc.vector.tensor_scalar_mul(gsq[:], gsq[:], inv_b)
            # var = E[x^2] - mean^2   (reuse gsq as var)
            m2 = sttp.tile([1, cfg.OC], F32, tag="m2")
            nc.vector.tensor_mul(out=m2[:], in0=gsum[:], in1=gsum[:])
            nc.vector.tensor_sub(out=gsq[:], in0=gsq[:], in1=m2[:])
            # istd = 1/sqrt(var+eps)  (m2 reused as std scratch)
            nc.scalar.activation(
                m2[:], gsq[:], mybir.ActivationFunctionType.Sqrt,
                bias=eps_t[0:1],
            )
            istd = sttp.tile([1, cfg.OC], F32, tag="istd")
            nc.vector.reciprocal(istd[:], m2[:])
            # shift = -mean*istd  (gsum reused)
            nc.vector.tensor_mul(out=gsum[:], in0=gsum[:], in1=istd[:])
            nc.vector.tensor_scalar_mul(gsum[:], gsum[:], -1.0)
            nc.gpsimd.dma_start(rows_d.ap()[oc, 0:1, :], istd[:])
            nc.gpsimd.dma_start(rows_d.ap()[oc, 1:2, :], gsum[:])

        def normalize(oc):
            scaleB = bcp.tile([P, cfg.OC], F32, tag="scaleB")
            nc.gpsimd.dma_start(
                scaleB[:], rows_d.ap()[oc, 0:1, :].broadcast_to([P, cfg.OC])
            )
            shiftB = bcp.tile([P, cfg.OC], F32, tag="shiftB")
            nc.gpsimd.dma_start(
                shiftB[:], rows_d.ap()[oc, 1:2, :].broadcast_to([P, cfg.OC])
            )
            sl = slice(oc * cfg.OC, (oc + 1) * cfg.OC)
            for bt in range(cfg.BT):
                raw = osbp.tile([P, cfg.OC], F16, tag="raw")
                nc.scalar.dma_start(
                    raw[:], raw_d.ap()[bt * P:(bt + 1) * P, sl]
                )
                t = normp.tile([P, cfg.OC], F32, tag="norm")
                nc.vector.tensor_mul(out=t[:], in0=raw[:], in1=scaleB[:])
                nc.vector.tensor_add(out=t[:], in0=t[:], in1=shiftB[:])
                nc.gpsimd.dma_start(y_out.ap()[bt * P:(bt + 1) * P, sl], t[:])

        # ---- software-pipelined chunk loop (prefetch 2 chunks) ----
        pre = {0: w_fetch(0)}
        if cfg.NOC > 1:
            pre[1] = w_fetch(1)
        for oc in range(cfg.NOC):
            bwt_cur = pre.pop(oc)
            if oc + 2 < cfg.NOC:
                pre[oc + 2] = w_fetch(oc + 2)
            psum_sum, psum_sq = matmuls(oc, bwt_cur)
            stats_copy(oc, psum_sum, psum_sq)
            if oc % 2 == 1 and oc < cfg.NOC - 1:
                ar_pair(oc - 1, 2)
            if oc >= 3 and oc % 2 == 1:
                for o2 in (oc - 3, oc - 2):
                    stats_math(o2)
                    normalize(o2)
        ar_pair(cfg.NOC - 2, 2)
        for o2 in range(cfg.NOC - 2, cfg.NOC):
            stats_math(o2)
            normalize(o2)


_CACHE = {}


def _get_program(reps: int = 1):
    if reps not in _CACHE:
        _CACHE[reps] = build_program(Cfg(), reps=reps)
    return _CACHE[reps]


def kernel(x, weight, bias=None):
    cfg = Cfg()
    x = np.asarray(x, dtype=np.float32)
    weight = np.asarray(weight, dtype=np.float32)
    assert x.shape == (cfg.B, cfg.IN) and weight.shape == (cfg.OUT, cfg.IN)

    nc = _get_program()
    in_maps = [
        {
            "x_shard": np.ascontiguousarray(x[c * cfg.B_SH:(c + 1) * cfg.B_SH]),
            "w_slice": np.ascontiguousarray(weight[c * cfg.OC:(c + 1) * cfg.OC]),
        }
        for c in range(cfg.n_cores)
    ]
    res = run_bass_kernel_spmd(nc, in_maps, core_ids=list(range(cfg.n_cores)))
    out = np.concatenate([res.results[c]["y"] for c in range(cfg.n_cores)], axis=0)
    return out.astype(np.float32)

